# revision 10
# baseline (speedup 1.0000x reference)
"""Trainium2 Bass kernel: batch row-sharded grouped GEMM (MoE routing).

Contract: kernel(x, weight, num_inputs_per_group) takes FULL inputs
  x (32768, 2048) f32, weight (16, 2048, 2048) f32, num_inputs_per_group (16,) i32
and returns the FULL output (32768, 2048) f32, where token row i is multiplied
by weight[seg[i]] with seg = repeat(arange(16), num, total_repeat_length=32768)
(contiguous groups).

Distribution strategy (no collectives needed): tokens are split into contiguous
128-row blocks; each of the 8 cores gets an equal number of blocks plus the
weights for the experts its blocks use (expert/token parallelism — sanctioned
by the sharding hint since E=16 >= 8). Each core computes a dense grouped GEMM
locally and the host concatenates the per-core outputs.

Device kernel (bf16 inputs, fp32 PSUM accumulation): the host rounds x and w
to bf16 (rel err ~1.6e-3, far under the 2e-2 gate), which halves HBM traffic
vs fp32r and lets a core keep a whole expert's weights (8 MiB) resident in
SBUF. Per expert run the kernel makes 4 n-major sweeps (one per 512-col
output set) over the run's token blocks with the x tiles resident, so x is
streamed exactly once and each 2 MiB weight n-set has a whole sweep (~55us)
of prefetch slack — the PE only ever waits for the first ~3 MiB at launch.
Weights ride the Sync-engine DMA ring; x and outputs ride the Scalar ring
with x tiles front-loaded per sub-run so output writes can never delay them.
"""

import sys

sys.path.insert(0, "/opt/trn_rl_repo")

import numpy as np

try:
    import ml_dtypes

    BF16 = np.dtype(ml_dtypes.bfloat16)
except Exception:  # pragma: no cover
    BF16 = None

import concourse.bacc as bacc
import concourse.mybir as mybir
from concourse.bass_utils import run_bass_kernel_spmd
from concourse.tile import TileContext

N_TOK, D_IN, D_OUT, N_EXP = 32768, 2048, 2048, 16
NCORES = 8
PB = 128  # token block = PSUM partition count
NT = 512  # matmul moving free dim = one fp32 PSUM bank
KT = D_IN // PB  # 16 k-tiles
NTILES = D_OUT // NT  # 4 output column sets
MG_BLOCKS = 2  # token blocks per x group tile
MGT = MG_BLOCKS * PB  # tokens per group tile
SUBRUN = 8  # max x group tiles held resident per n-major sweep set

# Introspection hooks for test.py (harness just calls kernel()).
TRACE = False
LAST_RESULTS = None


def _seg_from_groups(num):
    """Replicate jnp.repeat(arange(E), num, total_repeat_length=N) semantics."""
    num = np.asarray(num, dtype=np.int64)
    reps = np.repeat(np.arange(N_EXP, dtype=np.int32), np.maximum(num, 0))
    if len(reps) >= N_TOK:
        return reps[:N_TOK]
    pad = reps[-1] if len(reps) else np.int32(0)
    return np.concatenate([reps, np.full(N_TOK - len(reps), pad, np.int32)])


def _run_groups(runs):
    """Split each run's blocks into m-groups of up to MG_BLOCKS blocks."""
    groups = []  # (run_idx, g_blocks)
    for ri, (_, nb) in enumerate(runs):
        b = 0
        while b < nb:
            g = min(MG_BLOCKS, nb - b)
            groups.append((ri, g))
            b += g
    return groups


def _build_nc(n_blocks_core, runs, n_slots):
    """Build the per-core SPMD kernel.

    runs: list of (slot, n_blocks) with sum(n_blocks) == n_blocks_core.
    Every core runs this same program; per-core data (x slice, slot->expert
    weight choice) lives in the input maps.
    """
    T_core = n_blocks_core * PB
    f32 = mybir.dt.float32
    bf16 = mybir.dt.bfloat16
    groups = _run_groups(runs)
    # first block index of each group (global within the core)
    block_of = []
    b = 0
    for _, g in groups:
        block_of.append(b)
        b += g
    run_groups = [[] for _ in runs]  # per run: list of (global gi, g)
    for gi, (ri, g) in enumerate(groups):
        run_groups[ri].append((gi, g))

    nc = bacc.Bacc("TRN2", target_bir_lowering=False, debug=False, num_devices=NCORES)
    xh = nc.dram_tensor("xh", [len(groups), PB, KT, MGT], bf16, kind="ExternalInput")
    w = nc.dram_tensor("w", [n_slots, NTILES, PB, KT, NT], bf16, kind="ExternalInput")
    out = nc.dram_tensor("out", [T_core, D_OUT], f32, kind="ExternalOutput")

    with TileContext(nc) as tc:
        with (
            tc.tile_pool(name="wpool", bufs=5) as wpool,
            tc.tile_pool(name="xpool", bufs=SUBRUN + 2) as xpool,
            tc.tile_pool(name="opool", bufs=12) as opool,
            tc.tile_pool(name="warmsrc", bufs=1) as warmsrc,
            tc.tile_pool(name="pspool", bufs=7, space="PSUM") as pspool,
            tc.tile_pool(name="warmpool", bufs=1, space="PSUM") as warmpool,
        ):
            # Warm-up: ~30 throwaway matmuls keep the PE busy (and its HAM
            # clock gate at 8/8) through the input-ready barrier + first-DMA
            # window. fp32 (not fp32r) so the memset-produced scratch needs
            # no fp32r rounding producer.
            wsrc = warmsrc.tile([PB, PB], f32, name="warm_src", tag="warm")
            nc.vector.memset(wsrc, 0.0)
            wps = warmpool.tile([PB, PB], f32, name="warm_ps", tag="warm_ps")
            for _ in range(20):
                nc.tensor.matmul(wps, wsrc, wsrc, start=True, stop=True)

            xt = {}  # global gi -> live tile

            def emit_x_dma(gi, split, eng=None):
                t = xpool.tile([PB, KT, MGT], bf16, name=f"xt_{gi}", tag="xt")
                kh = KT // split
                for q in range(split):
                    (eng or nc.scalar).dma_start(
                        out=t[:, q * kh : (q + 1) * kh, :],
                        in_=xh[gi, :, q * kh : (q + 1) * kh, :],
                    )
                xt[gi] = t

            for ri, (slot, nb) in enumerate(runs):
                if ri == 0:
                    # Launch-critical pieces ride the Sync ring, which the
                    # runtime arms ~4us before the Scalar ring: the first x
                    # tile's k-lower half goes ahead of the weights, the
                    # first weight n-set is split into k-eighths so each
                    # ~0.25 MiB arrival unlocks more of the interleaved
                    # opening chains, and the x upper half leads the Scalar
                    # ring.
                    t = xpool.tile([PB, KT, MGT], bf16, name="xt_0", tag="xt")
                    nc.sync.dma_start(
                        out=t[:, : KT // 2, :], in_=xh[0, :, : KT // 2, :]
                    )
                    nc.scalar.dma_start(
                        out=t[:, KT // 2 :, :], in_=xh[0, :, KT // 2 :, :]
                    )
                    xt[0] = t
                # This run's weights: 4 n-set tiles, resident for the whole
                # run; each n-set has a whole sweep of prefetch slack.
                wt = []
                for n in range(NTILES):
                    t = wpool.tile([PB, KT, NT], bf16, name=f"w_r{ri}_n{n}", tag="w")
                    split = 4 if (ri == 0 and n == 0) else 2
                    kh = KT // split
                    for q in range(split):
                        nc.sync.dma_start(
                            out=t[:, q * kh : (q + 1) * kh, :],
                            in_=w[slot, n, :, q * kh : (q + 1) * kh, :],
                        )
                    wt.append(t)

                rg = run_groups[ri]
                for s0 in range(0, len(rg), SUBRUN):
                    sub = rg[s0 : s0 + SUBRUN]
                    last_sub = s0 + SUBRUN >= len(rg)
                    for sj, (gi, g) in enumerate(sub):
                        if gi not in xt:
                            # Last two tiles of the first sub-run ride the
                            # GpSimd ring: plenty of arrival slack there, and
                            # it relieves the Scalar ring during the ramp.
                            eng = (
                                nc.gpsimd
                                if (ri == 0 and s0 == 0 and sj >= len(sub) - 2)
                                else None
                            )
                            emit_x_dma(gi, split=2 if gi == 0 else 1, eng=eng)
                    if ri == 0 and s0 == 0:
                        # Opening chains for the first x tile, interleaved
                        # k-pair by k-pair across the tile's blocks so each
                        # arriving 0.25 MiB weight piece feeds g blocks'
                        # worth of matmuls instead of one chain's.
                        g0_g = sub[0][1]
                        pss = [
                            pspool.tile([PB, NT], f32, name="ps", tag="ps")
                            for _ in range(g0_g)
                        ]
                        for kp in range(0, KT, 2):
                            for mb in range(g0_g):
                                for k in (kp, kp + 1):
                                    nc.tensor.matmul(
                                        pss[mb],
                                        xt[0][:, k, mb * PB : (mb + 1) * PB],
                                        wt[0][:, k, :],
                                        start=(k == 0),
                                        stop=(k == KT - 1),
                                    )
                        for mb in range(g0_g):
                            ot = opool.tile([PB, NT], f32, name="o", tag="o")
                            nc.vector.tensor_copy(out=ot, in_=pss[mb])
                            row = (block_of[sub[0][0]] + mb) * PB
                            nc.scalar.dma_start(
                                out=out[row : row + PB, 0:NT], in_=ot
                            )
                    for n in range(NTILES):
                        # Before the final sweep of this run's last sub-run,
                        # prefetch the next run's first two x tiles so the
                        # run boundary never waits on the x stream.
                        if n == NTILES - 1 and last_sub and ri + 1 < len(runs):
                            for gi, _ in run_groups[ri + 1][:2]:
                                emit_x_dma(gi, split=1)
                        for sj, (gi, g) in enumerate(sub):
                            if ri == 0 and s0 == 0 and n == 0 and sj == 0:
                                continue  # opening chains already emitted
                            for mb in range(g):
                                ps = pspool.tile([PB, NT], f32, name="ps", tag="ps")
                                for k in range(KT):
                                    nc.tensor.matmul(
                                        ps,
                                        xt[gi][:, k, mb * PB : (mb + 1) * PB],
                                        wt[n][:, k, :],
                                        start=(k == 0),
                                        stop=(k == KT - 1),
                                    )
                                row = (block_of[gi] + mb) * PB
                                final = (
                                    ri == len(runs) - 1
                                    and last_sub
                                    and n == NTILES - 1
                                    and sj == len(sub) - 1
                                    and mb == g - 1
                                )
                                if final:
                                    # Split the very last output across both
                                    # DMA rings so the post-matmul tail is
                                    # half a copy + two parallel 128 KiB
                                    # writes instead of one serial 256 KiB.
                                    nh = NT // 2
                                    for h, eng in ((0, nc.scalar), (1, nc.sync)):
                                        ot = opool.tile(
                                            [PB, nh], f32, name="o", tag="o"
                                        )
                                        nc.vector.tensor_copy(
                                            out=ot, in_=ps[:, h * nh : (h + 1) * nh]
                                        )
                                        col = n * NT + h * nh
                                        eng.dma_start(
                                            out=out[row : row + PB, col : col + nh],
                                            in_=ot,
                                        )
                                else:
                                    ot = opool.tile([PB, NT], f32, name="o", tag="o")
                                    nc.vector.tensor_copy(out=ot, in_=ps)
                                    nc.scalar.dma_start(
                                        out=out[
                                            row : row + PB, n * NT : (n + 1) * NT
                                        ],
                                        in_=ot,
                                    )
                    # Tiles of a finished sub-run are recycled by the pool.
                    for gi, g in sub:
                        del xt[gi]
    nc.compile()
    return nc


def _host_layout_x(x_core, runs):
    """Pack a core's bf16 tokens [T, D] into group tiles [NG, 128, 16, 256]."""
    groups = _run_groups(runs)
    xh = np.zeros((len(groups), PB, KT, MGT), dtype=BF16)
    t0 = 0
    for i, (_, g) in enumerate(groups):
        gt = g * PB
        blockT = x_core[t0 : t0 + gt]  # [gt, D]
        # (t, k, p) -> (p, k, t)
        xh[i, :, :, :gt] = blockT.reshape(gt, KT, PB).transpose(2, 1, 0)
        t0 += gt
    return np.ascontiguousarray(xh)


def _host_layout_w(w_slots):
    """Pack bf16 slot weights [S, D, O] into n-set tiles [S, 4, 128, 16, 512]."""
    S = w_slots.shape[0]
    # (s, k, p, n, j) -> (s, n, p, k, j)
    return np.ascontiguousarray(
        w_slots.reshape(S, KT, PB, NTILES, NT).transpose(0, 3, 2, 1, 4)
    )


def kernel(x, weight, num_inputs_per_group):
    global LAST_RESULTS
    x = np.asarray(x, dtype=np.float32)
    weight = np.asarray(weight, dtype=np.float32)
    seg = _seg_from_groups(num_inputs_per_group)
    x_bf = np.ascontiguousarray(x.astype(BF16))
    w_bf = np.ascontiguousarray(weight.astype(BF16))

    # --- plan: map 128-token blocks to experts ---------------------------------
    aligned = all(
        np.all(seg[i * PB : (i + 1) * PB] == seg[i * PB]) for i in range(N_TOK // PB)
    )
    if aligned:
        block_expert = seg[::PB].astype(np.int64)  # (256,)
        block_tokens = None  # identity: block b covers rows [b*128, (b+1)*128)
    else:
        # Generic fallback: pad each contiguous expert segment to a 128 multiple
        # via a host-side gather; output rows are scattered back afterwards.
        bounds = np.flatnonzero(np.diff(seg)) + 1
        starts = np.concatenate([[0], bounds])
        ends = np.concatenate([bounds, [N_TOK]])
        blocks, experts = [], []
        for s, e in zip(starts, ends):
            idx = np.arange(s, e, dtype=np.int64)
            padded = -np.ones(int(np.ceil(len(idx) / PB)) * PB, dtype=np.int64)
            padded[: len(idx)] = idx
            for b0 in range(0, len(padded), PB):
                blocks.append(padded[b0 : b0 + PB])
                experts.append(int(seg[s]))
        while len(blocks) % NCORES:
            blocks.append(-np.ones(PB, dtype=np.int64))
            experts.append(0)
        block_tokens = np.stack(blocks)  # (n_blocks, 128) token ids, -1 = pad
        block_expert = np.asarray(experts, dtype=np.int64)

    n_blocks = len(block_expert)
    n_blocks_core = n_blocks // NCORES
    per_core_experts = block_expert.reshape(NCORES, n_blocks_core)

    # Run-length encode each core's block->expert map; if all cores share the
    # same run-length pattern we can use compact per-run weight slots.
    def rle(v):
        runs = []
        for e in v:
            if runs and runs[-1][0] == e:
                runs[-1][1] += 1
            else:
                runs.append([int(e), 1])
        return runs

    core_runs = [rle(per_core_experts[c]) for c in range(NCORES)]
    lengths0 = [n for _, n in core_runs[0]]
    if all([n for _, n in core_runs[c]] == lengths0 for c in range(NCORES)):
        runs = [(s, n) for s, (_, n) in enumerate(core_runs[0])]
        slot_experts = [[e for e, _ in core_runs[c]] for c in range(NCORES)]
    else:
        runs = [(b, 1) for b in range(n_blocks_core)]
        slot_experts = [list(per_core_experts[c]) for c in range(NCORES)]
    n_slots = len(runs)

    # --- per-core inputs -------------------------------------------------------
    in_maps = []
    for c in range(NCORES):
        if block_tokens is None:
            rows = slice(c * n_blocks_core * PB, (c + 1) * n_blocks_core * PB)
            xc = x_bf[rows]
        else:
            tok = block_tokens[c * n_blocks_core : (c + 1) * n_blocks_core].ravel()
            xc = np.where(
                tok[:, None] >= 0, x_bf[np.maximum(tok, 0)], np.zeros((), BF16)
            ).astype(BF16)
        in_maps.append(
            {
                "xh": _host_layout_x(xc, runs),
                "w": _host_layout_w(w_bf[slot_experts[c]]),
            }
        )

    nc = _build_nc(n_blocks_core, runs, n_slots)
    res = run_bass_kernel_spmd(nc, in_maps, core_ids=list(range(NCORES)), trace=TRACE)
    LAST_RESULTS = res

    # --- unshard ---------------------------------------------------------------
    outs = [res.results[c]["out"] for c in range(NCORES)]
    if block_tokens is None:
        return np.concatenate(outs, axis=0)
    full = np.zeros((N_TOK, D_OUT), dtype=np.float32)
    flat_tok = block_tokens.ravel()
    flat_out = np.concatenate(outs, axis=0)
    valid = flat_tok >= 0
    full[flat_tok[valid]] = flat_out[valid]
    return full


# revision 14
# speedup vs baseline: 1.0266x; 1.0266x over previous
"""Trainium2 Bass kernel: batch row-sharded grouped GEMM (MoE routing).

Contract: kernel(x, weight, num_inputs_per_group) takes FULL inputs
  x (32768, 2048) f32, weight (16, 2048, 2048) f32, num_inputs_per_group (16,) i32
and returns the FULL output (32768, 2048) f32, where token row i is multiplied
by weight[seg[i]] with seg = repeat(arange(16), num, total_repeat_length=32768)
(contiguous groups).

Distribution strategy (no collectives needed): tokens are split into contiguous
128-row blocks; each of the 8 cores gets an equal number of blocks plus the
weights for the experts its blocks use (expert/token parallelism — sanctioned
by the sharding hint since E=16 >= 8). Each core computes a dense grouped GEMM
locally and the host concatenates the per-core outputs.

Device kernel (bf16 inputs, fp32 PSUM accumulation): the host rounds x and w
to bf16 (rel err ~1.6e-3, far under the 2e-2 gate), which halves HBM traffic
vs fp32r and lets a core keep a whole expert's weights (8 MiB) resident in
SBUF. Per expert run the kernel makes 4 n-major sweeps (one per 512-col
output set) over the run's token blocks with the x tiles resident, so x is
streamed exactly once and each 2 MiB weight n-set has a whole sweep (~55us)
of prefetch slack — the PE only ever waits for the first ~3 MiB at launch.
Weights ride the Sync-engine DMA ring; x and outputs ride the Scalar ring
with x tiles front-loaded per sub-run so output writes can never delay them.
"""

import sys

sys.path.insert(0, "/opt/trn_rl_repo")

import numpy as np

try:
    import ml_dtypes

    BF16 = np.dtype(ml_dtypes.bfloat16)
except Exception:  # pragma: no cover
    BF16 = None

import concourse.bacc as bacc
import concourse.mybir as mybir
from concourse.bass_utils import run_bass_kernel_spmd
from concourse.tile import TileContext

N_TOK, D_IN, D_OUT, N_EXP = 32768, 2048, 2048, 16
NCORES = 8
PB = 128  # token block = PSUM partition count
NT = 512  # matmul moving free dim = one fp32 PSUM bank
KT = D_IN // PB  # 16 k-tiles
NTILES = D_OUT // NT  # 4 output column sets
MG_BLOCKS = 2  # token blocks per x group tile
MGT = MG_BLOCKS * PB  # tokens per group tile
SUBRUN = 8  # max x group tiles held resident per n-major sweep set

# Introspection hooks for test.py (harness just calls kernel()).
TRACE = False
LAST_RESULTS = None


def _seg_from_groups(num):
    """Replicate jnp.repeat(arange(E), num, total_repeat_length=N) semantics."""
    num = np.asarray(num, dtype=np.int64)
    reps = np.repeat(np.arange(N_EXP, dtype=np.int32), np.maximum(num, 0))
    if len(reps) >= N_TOK:
        return reps[:N_TOK]
    pad = reps[-1] if len(reps) else np.int32(0)
    return np.concatenate([reps, np.full(N_TOK - len(reps), pad, np.int32)])


def _run_groups(runs):
    """Split each run's blocks into m-groups of up to MG_BLOCKS blocks."""
    groups = []  # (run_idx, g_blocks)
    for ri, (_, nb) in enumerate(runs):
        b = 0
        while b < nb:
            g = min(MG_BLOCKS, nb - b)
            groups.append((ri, g))
            b += g
    return groups


def _build_nc(n_blocks_core, runs, n_slots):
    """Build the per-core SPMD kernel.

    runs: list of (slot, n_blocks) with sum(n_blocks) == n_blocks_core.
    Every core runs this same program; per-core data (x slice, slot->expert
    weight choice) lives in the input maps.
    """
    T_core = n_blocks_core * PB
    f32 = mybir.dt.float32
    bf16 = mybir.dt.bfloat16
    groups = _run_groups(runs)
    # first block index of each group (global within the core)
    block_of = []
    b = 0
    for _, g in groups:
        block_of.append(b)
        b += g
    run_groups = [[] for _ in runs]  # per run: list of (global gi, g)
    for gi, (ri, g) in enumerate(groups):
        run_groups[ri].append((gi, g))

    nc = bacc.Bacc("TRN2", target_bir_lowering=False, debug=False, num_devices=NCORES)
    xh = nc.dram_tensor("xh", [len(groups), PB, KT, MGT], bf16, kind="ExternalInput")
    w = nc.dram_tensor("w", [n_slots, NTILES, PB, KT, NT], bf16, kind="ExternalInput")
    out = nc.dram_tensor("out", [T_core, D_OUT], f32, kind="ExternalOutput")

    with TileContext(nc) as tc:
        with (
            tc.tile_pool(name="wpool", bufs=5) as wpool,
            tc.tile_pool(name="xpool", bufs=SUBRUN + 2) as xpool,
            tc.tile_pool(name="opool", bufs=12) as opool,
            tc.tile_pool(name="warmsrc", bufs=1) as warmsrc,
            tc.tile_pool(name="pspool", bufs=7, space="PSUM") as pspool,
            tc.tile_pool(name="warmpool", bufs=1, space="PSUM") as warmpool,
        ):
            # Warm-up: ~30 throwaway matmuls keep the PE busy (and its HAM
            # clock gate at 8/8) through the input-ready barrier + first-DMA
            # window. fp32 (not fp32r) so the memset-produced scratch needs
            # no fp32r rounding producer.
            wsrc = warmsrc.tile([PB, PB], f32, name="warm_src", tag="warm")
            nc.vector.memset(wsrc, 0.0)
            wps = warmpool.tile([PB, PB], f32, name="warm_ps", tag="warm_ps")
            for _ in range(15):
                nc.tensor.matmul(wps, wsrc, wsrc, start=True, stop=True)

            xt = {}  # global gi -> live tile

            def emit_x_dma(gi, split, eng=None):
                t = xpool.tile([PB, KT, MGT], bf16, name=f"xt_{gi}", tag="xt")
                kh = KT // split
                for q in range(split):
                    (eng or nc.scalar).dma_start(
                        out=t[:, q * kh : (q + 1) * kh, :],
                        in_=xh[gi, :, q * kh : (q + 1) * kh, :],
                    )
                xt[gi] = t

            for ri, (slot, nb) in enumerate(runs):
                x0 = None
                if ri == 0:
                    # Launch ramp: deliver bytes in strict need order across
                    # both rings in parallel — the first x tile's k-lower
                    # half leads the Sync ring while weight quarter 0 leads
                    # the Scalar ring; the x upper half rides Scalar right
                    # behind quarter 0, and quarters 1-3 follow on Sync.
                    x0 = xpool.tile([PB, KT, MGT], bf16, name="xt_0", tag="xt")
                    nc.sync.dma_start(
                        out=x0[:, : KT // 2, :], in_=xh[0, :, : KT // 2, :]
                    )
                    xt[0] = x0
                # This run's weights: 4 n-set tiles, resident for the whole
                # run; each n-set has a whole sweep of prefetch slack.
                wt = []
                for n in range(NTILES):
                    t = wpool.tile([PB, KT, NT], bf16, name=f"w_r{ri}_n{n}", tag="w")
                    split = 4 if (ri == 0 and n == 0) else 2
                    kh = KT // split
                    for q in range(split):
                        eng = nc.scalar if (split == 4 and q == 0) else nc.sync
                        eng.dma_start(
                            out=t[:, q * kh : (q + 1) * kh, :],
                            in_=w[slot, n, :, q * kh : (q + 1) * kh, :],
                        )
                    wt.append(t)
                if x0 is not None:
                    nc.scalar.dma_start(
                        out=x0[:, KT // 2 :, :], in_=xh[0, :, KT // 2 :, :]
                    )

                rg = run_groups[ri]
                for s0 in range(0, len(rg), SUBRUN):
                    sub = rg[s0 : s0 + SUBRUN]
                    last_sub = s0 + SUBRUN >= len(rg)
                    for sj, (gi, g) in enumerate(sub):
                        if gi not in xt:
                            emit_x_dma(gi, split=1)
                    if ri == 0 and s0 == 0:
                        # Opening chains for the first x tile, interleaved
                        # k-pair by k-pair across the tile's blocks so each
                        # arriving 0.25 MiB weight piece feeds g blocks'
                        # worth of matmuls instead of one chain's.
                        g0_g = sub[0][1]
                        pss = [
                            pspool.tile([PB, NT], f32, name="ps", tag="ps")
                            for _ in range(g0_g)
                        ]
                        for kp in range(0, KT, 2):
                            for mb in range(g0_g):
                                for k in (kp, kp + 1):
                                    nc.tensor.matmul(
                                        pss[mb],
                                        xt[0][:, k, mb * PB : (mb + 1) * PB],
                                        wt[0][:, k, :],
                                        start=(k == 0),
                                        stop=(k == KT - 1),
                                    )
                        for mb in range(g0_g):
                            ot = opool.tile([PB, NT], f32, name="o", tag="o")
                            nc.vector.tensor_copy(out=ot, in_=pss[mb])
                            row = (block_of[sub[0][0]] + mb) * PB
                            nc.scalar.dma_start(
                                out=out[row : row + PB, 0:NT], in_=ot
                            )
                    for n in range(NTILES):
                        # Before the final sweep of this run's last sub-run,
                        # prefetch the next run's first two x tiles so the
                        # run boundary never waits on the x stream.
                        if n == NTILES - 1 and last_sub and ri + 1 < len(runs):
                            for gi, _ in run_groups[ri + 1][:2]:
                                emit_x_dma(gi, split=1)
                        for sj, (gi, g) in enumerate(sub):
                            if ri == 0 and s0 == 0 and n == 0 and sj == 0:
                                continue  # opening chains already emitted
                            for mb in range(g):
                                ps = pspool.tile([PB, NT], f32, name="ps", tag="ps")
                                for k in range(KT):
                                    nc.tensor.matmul(
                                        ps,
                                        xt[gi][:, k, mb * PB : (mb + 1) * PB],
                                        wt[n][:, k, :],
                                        start=(k == 0),
                                        stop=(k == KT - 1),
                                    )
                                row = (block_of[gi] + mb) * PB
                                ot = opool.tile([PB, NT], f32, name="o", tag="o")
                                nc.vector.tensor_copy(out=ot, in_=ps)
                                nc.scalar.dma_start(
                                    out=out[row : row + PB, n * NT : (n + 1) * NT],
                                    in_=ot,
                                )
                    # Tiles of a finished sub-run are recycled by the pool.
                    for gi, g in sub:
                        del xt[gi]
    nc.compile()
    return nc


def _host_layout_x(x_core, runs):
    """Pack a core's bf16 tokens [T, D] into group tiles [NG, 128, 16, 256]."""
    groups = _run_groups(runs)
    xh = np.zeros((len(groups), PB, KT, MGT), dtype=BF16)
    t0 = 0
    for i, (_, g) in enumerate(groups):
        gt = g * PB
        blockT = x_core[t0 : t0 + gt]  # [gt, D]
        # (t, k, p) -> (p, k, t)
        xh[i, :, :, :gt] = blockT.reshape(gt, KT, PB).transpose(2, 1, 0)
        t0 += gt
    return np.ascontiguousarray(xh)


def _host_layout_w(w_slots):
    """Pack bf16 slot weights [S, D, O] into n-set tiles [S, 4, 128, 16, 512]."""
    S = w_slots.shape[0]
    # (s, k, p, n, j) -> (s, n, p, k, j)
    return np.ascontiguousarray(
        w_slots.reshape(S, KT, PB, NTILES, NT).transpose(0, 3, 2, 1, 4)
    )


def kernel(x, weight, num_inputs_per_group):
    global LAST_RESULTS
    x = np.asarray(x, dtype=np.float32)
    weight = np.asarray(weight, dtype=np.float32)
    seg = _seg_from_groups(num_inputs_per_group)
    x_bf = np.ascontiguousarray(x.astype(BF16))
    w_bf = np.ascontiguousarray(weight.astype(BF16))

    # --- plan: map 128-token blocks to experts ---------------------------------
    aligned = all(
        np.all(seg[i * PB : (i + 1) * PB] == seg[i * PB]) for i in range(N_TOK // PB)
    )
    if aligned:
        block_expert = seg[::PB].astype(np.int64)  # (256,)
        block_tokens = None  # identity: block b covers rows [b*128, (b+1)*128)
    else:
        # Generic fallback: pad each contiguous expert segment to a 128 multiple
        # via a host-side gather; output rows are scattered back afterwards.
        bounds = np.flatnonzero(np.diff(seg)) + 1
        starts = np.concatenate([[0], bounds])
        ends = np.concatenate([bounds, [N_TOK]])
        blocks, experts = [], []
        for s, e in zip(starts, ends):
            idx = np.arange(s, e, dtype=np.int64)
            padded = -np.ones(int(np.ceil(len(idx) / PB)) * PB, dtype=np.int64)
            padded[: len(idx)] = idx
            for b0 in range(0, len(padded), PB):
                blocks.append(padded[b0 : b0 + PB])
                experts.append(int(seg[s]))
        while len(blocks) % NCORES:
            blocks.append(-np.ones(PB, dtype=np.int64))
            experts.append(0)
        block_tokens = np.stack(blocks)  # (n_blocks, 128) token ids, -1 = pad
        block_expert = np.asarray(experts, dtype=np.int64)

    n_blocks = len(block_expert)
    n_blocks_core = n_blocks // NCORES
    per_core_experts = block_expert.reshape(NCORES, n_blocks_core)

    # Run-length encode each core's block->expert map; if all cores share the
    # same run-length pattern we can use compact per-run weight slots.
    def rle(v):
        runs = []
        for e in v:
            if runs and runs[-1][0] == e:
                runs[-1][1] += 1
            else:
                runs.append([int(e), 1])
        return runs

    core_runs = [rle(per_core_experts[c]) for c in range(NCORES)]
    lengths0 = [n for _, n in core_runs[0]]
    if all([n for _, n in core_runs[c]] == lengths0 for c in range(NCORES)):
        runs = [(s, n) for s, (_, n) in enumerate(core_runs[0])]
        slot_experts = [[e for e, _ in core_runs[c]] for c in range(NCORES)]
    else:
        runs = [(b, 1) for b in range(n_blocks_core)]
        slot_experts = [list(per_core_experts[c]) for c in range(NCORES)]
    n_slots = len(runs)

    # --- per-core inputs -------------------------------------------------------
    in_maps = []
    for c in range(NCORES):
        if block_tokens is None:
            rows = slice(c * n_blocks_core * PB, (c + 1) * n_blocks_core * PB)
            xc = x_bf[rows]
        else:
            tok = block_tokens[c * n_blocks_core : (c + 1) * n_blocks_core].ravel()
            xc = np.where(
                tok[:, None] >= 0, x_bf[np.maximum(tok, 0)], np.zeros((), BF16)
            ).astype(BF16)
        in_maps.append(
            {
                "xh": _host_layout_x(xc, runs),
                "w": _host_layout_w(w_bf[slot_experts[c]]),
            }
        )

    nc = _build_nc(n_blocks_core, runs, n_slots)
    res = run_bass_kernel_spmd(nc, in_maps, core_ids=list(range(NCORES)), trace=TRACE)
    LAST_RESULTS = res

    # --- unshard ---------------------------------------------------------------
    outs = [res.results[c]["out"] for c in range(NCORES)]
    if block_tokens is None:
        return np.concatenate(outs, axis=0)
    full = np.zeros((N_TOK, D_OUT), dtype=np.float32)
    flat_tok = block_tokens.ravel()
    flat_out = np.concatenate(outs, axis=0)
    valid = flat_tok >= 0
    full[flat_tok[valid]] = flat_out[valid]
    return full


# revision 15
# speedup vs baseline: 1.0292x; 1.0025x over previous
"""Trainium2 Bass kernel: batch row-sharded grouped GEMM (MoE routing).

Contract: kernel(x, weight, num_inputs_per_group) takes FULL inputs
  x (32768, 2048) f32, weight (16, 2048, 2048) f32, num_inputs_per_group (16,) i32
and returns the FULL output (32768, 2048) f32, where token row i is multiplied
by weight[seg[i]] with seg = repeat(arange(16), num, total_repeat_length=32768)
(contiguous groups).

Distribution strategy (no collectives needed): tokens are split into contiguous
128-row blocks; each of the 8 cores gets an equal number of blocks plus the
weights for the experts its blocks use (expert/token parallelism — sanctioned
by the sharding hint since E=16 >= 8). Each core computes a dense grouped GEMM
locally and the host concatenates the per-core outputs.

Device kernel (bf16 inputs, fp32 PSUM accumulation): the host rounds x and w
to bf16 (rel err ~1.6e-3, far under the 2e-2 gate), which halves HBM traffic
vs fp32r and lets a core keep a whole expert's weights (8 MiB) resident in
SBUF. Per expert run the kernel makes 4 n-major sweeps (one per 512-col
output set) over the run's token blocks with the x tiles resident, so x is
streamed exactly once and each 2 MiB weight n-set has a whole sweep (~55us)
of prefetch slack — the PE only ever waits for the first ~3 MiB at launch.
Weights ride the Sync-engine DMA ring; x and outputs ride the Scalar ring
with x tiles front-loaded per sub-run so output writes can never delay them.
"""

import sys

sys.path.insert(0, "/opt/trn_rl_repo")

import numpy as np

try:
    import ml_dtypes

    BF16 = np.dtype(ml_dtypes.bfloat16)
except Exception:  # pragma: no cover
    BF16 = None

import concourse.bacc as bacc
import concourse.mybir as mybir
from concourse.bass_utils import run_bass_kernel_spmd
from concourse.tile import TileContext

N_TOK, D_IN, D_OUT, N_EXP = 32768, 2048, 2048, 16
NCORES = 8
PB = 128  # token block = PSUM partition count
NT = 512  # matmul moving free dim = one fp32 PSUM bank
KT = D_IN // PB  # 16 k-tiles
NTILES = D_OUT // NT  # 4 output column sets
MG_BLOCKS = 2  # token blocks per x group tile
MGT = MG_BLOCKS * PB  # tokens per group tile
SUBRUN = 8  # max x group tiles held resident per n-major sweep set

# Introspection hooks for test.py (harness just calls kernel()).
TRACE = False
LAST_RESULTS = None


def _seg_from_groups(num):
    """Replicate jnp.repeat(arange(E), num, total_repeat_length=N) semantics."""
    num = np.asarray(num, dtype=np.int64)
    reps = np.repeat(np.arange(N_EXP, dtype=np.int32), np.maximum(num, 0))
    if len(reps) >= N_TOK:
        return reps[:N_TOK]
    pad = reps[-1] if len(reps) else np.int32(0)
    return np.concatenate([reps, np.full(N_TOK - len(reps), pad, np.int32)])


def _run_groups(runs):
    """Split each run's blocks into m-groups of up to MG_BLOCKS blocks."""
    groups = []  # (run_idx, g_blocks)
    for ri, (_, nb) in enumerate(runs):
        b = 0
        while b < nb:
            g = min(MG_BLOCKS, nb - b)
            groups.append((ri, g))
            b += g
    return groups


def _build_nc(n_blocks_core, runs, n_slots):
    """Build the per-core SPMD kernel.

    runs: list of (slot, n_blocks) with sum(n_blocks) == n_blocks_core.
    Every core runs this same program; per-core data (x slice, slot->expert
    weight choice) lives in the input maps.
    """
    T_core = n_blocks_core * PB
    f32 = mybir.dt.float32
    bf16 = mybir.dt.bfloat16
    groups = _run_groups(runs)
    # first block index of each group (global within the core)
    block_of = []
    b = 0
    for _, g in groups:
        block_of.append(b)
        b += g
    run_groups = [[] for _ in runs]  # per run: list of (global gi, g)
    for gi, (ri, g) in enumerate(groups):
        run_groups[ri].append((gi, g))

    nc = bacc.Bacc("TRN2", target_bir_lowering=False, debug=False, num_devices=NCORES)
    xh = nc.dram_tensor("xh", [len(groups), PB, KT, MGT], bf16, kind="ExternalInput")
    w = nc.dram_tensor("w", [n_slots, NTILES, PB, KT, NT], bf16, kind="ExternalInput")
    out = nc.dram_tensor("out", [T_core, D_OUT], f32, kind="ExternalOutput")

    with TileContext(nc) as tc:
        with (
            tc.tile_pool(name="wpool", bufs=5) as wpool,
            tc.tile_pool(name="xpool", bufs=SUBRUN + 2) as xpool,
            tc.tile_pool(name="opool", bufs=12) as opool,
            tc.tile_pool(name="warmsrc", bufs=1) as warmsrc,
            tc.tile_pool(name="pspool", bufs=7, space="PSUM") as pspool,
            tc.tile_pool(name="warmpool", bufs=1, space="PSUM") as warmpool,
        ):
            # Warm-up: ~30 throwaway matmuls keep the PE busy (and its HAM
            # clock gate at 8/8) through the input-ready barrier + first-DMA
            # window. fp32 (not fp32r) so the memset-produced scratch needs
            # no fp32r rounding producer.
            wsrc = warmsrc.tile([PB, PB], f32, name="warm_src", tag="warm")
            nc.vector.memset(wsrc, 0.0)
            wps = warmpool.tile([PB, PB], f32, name="warm_ps", tag="warm_ps")
            for _ in range(15):
                nc.tensor.matmul(wps, wsrc, wsrc, start=True, stop=True)

            xt = {}  # global gi -> live tile

            def emit_x_dma(gi, split, eng=None):
                t = xpool.tile([PB, KT, MGT], bf16, name=f"xt_{gi}", tag="xt")
                kh = KT // split
                for q in range(split):
                    (eng or nc.scalar).dma_start(
                        out=t[:, q * kh : (q + 1) * kh, :],
                        in_=xh[gi, :, q * kh : (q + 1) * kh, :],
                    )
                xt[gi] = t

            for ri, (slot, nb) in enumerate(runs):
                wt = []
                if ri == 0:
                    # Launch ramp: deliver bytes in strict need order,
                    # alternating across both rings so arrival pace matches
                    # the opening chains' consumption.
                    #   Sync:   x0 k-lower, w q1, w q3, then n1..n3 halves
                    #   Scalar: w q0, x0 k-upper, w q2, then g1.. and outs
                    x0 = xpool.tile([PB, KT, MGT], bf16, name="xt_0", tag="xt")
                    w0 = wpool.tile([PB, KT, NT], bf16, name="w_r0_n0", tag="w")
                    kq = KT // 4
                    wq = lambda q, eng: eng.dma_start(
                        out=w0[:, q * kq : (q + 1) * kq, :],
                        in_=w[slot, 0, :, q * kq : (q + 1) * kq, :],
                    )
                    nc.sync.dma_start(
                        out=x0[:, : KT // 2, :], in_=xh[0, :, : KT // 2, :]
                    )
                    wq(0, nc.scalar)
                    wq(1, nc.sync)
                    nc.scalar.dma_start(
                        out=x0[:, KT // 2 :, :], in_=xh[0, :, KT // 2 :, :]
                    )
                    wq(2, nc.scalar)
                    wq(3, nc.sync)
                    xt[0] = x0
                    wt.append(w0)
                # This run's weights: 4 n-set tiles, resident for the whole
                # run; each n-set has a whole sweep of prefetch slack.
                for n in range(len(wt), NTILES):
                    t = wpool.tile([PB, KT, NT], bf16, name=f"w_r{ri}_n{n}", tag="w")
                    kh = KT // 2
                    for q in range(2):
                        nc.sync.dma_start(
                            out=t[:, q * kh : (q + 1) * kh, :],
                            in_=w[slot, n, :, q * kh : (q + 1) * kh, :],
                        )
                    wt.append(t)

                rg = run_groups[ri]
                for s0 in range(0, len(rg), SUBRUN):
                    sub = rg[s0 : s0 + SUBRUN]
                    last_sub = s0 + SUBRUN >= len(rg)
                    for sj, (gi, g) in enumerate(sub):
                        if gi not in xt:
                            emit_x_dma(gi, split=1)
                    if ri == 0 and s0 == 0:
                        # Opening chains for the first x tile, interleaved
                        # k-pair by k-pair across the tile's blocks so each
                        # arriving 0.25 MiB weight piece feeds g blocks'
                        # worth of matmuls instead of one chain's.
                        g0_g = sub[0][1]
                        pss = [
                            pspool.tile([PB, NT], f32, name="ps", tag="ps")
                            for _ in range(g0_g)
                        ]
                        for kp in range(0, KT, 2):
                            for mb in range(g0_g):
                                for k in (kp, kp + 1):
                                    nc.tensor.matmul(
                                        pss[mb],
                                        xt[0][:, k, mb * PB : (mb + 1) * PB],
                                        wt[0][:, k, :],
                                        start=(k == 0),
                                        stop=(k == KT - 1),
                                    )
                        for mb in range(g0_g):
                            ot = opool.tile([PB, NT], f32, name="o", tag="o")
                            nc.vector.tensor_copy(out=ot, in_=pss[mb])
                            row = (block_of[sub[0][0]] + mb) * PB
                            nc.scalar.dma_start(
                                out=out[row : row + PB, 0:NT], in_=ot
                            )
                    for n in range(NTILES):
                        # Before the final sweep of this run's last sub-run,
                        # prefetch the next run's first two x tiles so the
                        # run boundary never waits on the x stream.
                        if n == NTILES - 1 and last_sub and ri + 1 < len(runs):
                            for gi, _ in run_groups[ri + 1][:2]:
                                emit_x_dma(gi, split=1)
                        for sj, (gi, g) in enumerate(sub):
                            if ri == 0 and s0 == 0 and n == 0 and sj == 0:
                                continue  # opening chains already emitted
                            for mb in range(g):
                                ps = pspool.tile([PB, NT], f32, name="ps", tag="ps")
                                for k in range(KT):
                                    nc.tensor.matmul(
                                        ps,
                                        xt[gi][:, k, mb * PB : (mb + 1) * PB],
                                        wt[n][:, k, :],
                                        start=(k == 0),
                                        stop=(k == KT - 1),
                                    )
                                row = (block_of[gi] + mb) * PB
                                ot = opool.tile([PB, NT], f32, name="o", tag="o")
                                nc.vector.tensor_copy(out=ot, in_=ps)
                                nc.scalar.dma_start(
                                    out=out[row : row + PB, n * NT : (n + 1) * NT],
                                    in_=ot,
                                )
                    # Tiles of a finished sub-run are recycled by the pool.
                    for gi, g in sub:
                        del xt[gi]
    nc.compile()
    return nc


def _host_layout_x(x_core, runs):
    """Pack a core's bf16 tokens [T, D] into group tiles [NG, 128, 16, 256]."""
    groups = _run_groups(runs)
    xh = np.zeros((len(groups), PB, KT, MGT), dtype=BF16)
    t0 = 0
    for i, (_, g) in enumerate(groups):
        gt = g * PB
        blockT = x_core[t0 : t0 + gt]  # [gt, D]
        # (t, k, p) -> (p, k, t)
        xh[i, :, :, :gt] = blockT.reshape(gt, KT, PB).transpose(2, 1, 0)
        t0 += gt
    return np.ascontiguousarray(xh)


def _host_layout_w(w_slots):
    """Pack bf16 slot weights [S, D, O] into n-set tiles [S, 4, 128, 16, 512]."""
    S = w_slots.shape[0]
    # (s, k, p, n, j) -> (s, n, p, k, j)
    return np.ascontiguousarray(
        w_slots.reshape(S, KT, PB, NTILES, NT).transpose(0, 3, 2, 1, 4)
    )


def kernel(x, weight, num_inputs_per_group):
    global LAST_RESULTS
    x = np.asarray(x, dtype=np.float32)
    weight = np.asarray(weight, dtype=np.float32)
    seg = _seg_from_groups(num_inputs_per_group)
    x_bf = np.ascontiguousarray(x.astype(BF16))
    w_bf = np.ascontiguousarray(weight.astype(BF16))

    # --- plan: map 128-token blocks to experts ---------------------------------
    aligned = all(
        np.all(seg[i * PB : (i + 1) * PB] == seg[i * PB]) for i in range(N_TOK // PB)
    )
    if aligned:
        block_expert = seg[::PB].astype(np.int64)  # (256,)
        block_tokens = None  # identity: block b covers rows [b*128, (b+1)*128)
    else:
        # Generic fallback: pad each contiguous expert segment to a 128 multiple
        # via a host-side gather; output rows are scattered back afterwards.
        bounds = np.flatnonzero(np.diff(seg)) + 1
        starts = np.concatenate([[0], bounds])
        ends = np.concatenate([bounds, [N_TOK]])
        blocks, experts = [], []
        for s, e in zip(starts, ends):
            idx = np.arange(s, e, dtype=np.int64)
            padded = -np.ones(int(np.ceil(len(idx) / PB)) * PB, dtype=np.int64)
            padded[: len(idx)] = idx
            for b0 in range(0, len(padded), PB):
                blocks.append(padded[b0 : b0 + PB])
                experts.append(int(seg[s]))
        while len(blocks) % NCORES:
            blocks.append(-np.ones(PB, dtype=np.int64))
            experts.append(0)
        block_tokens = np.stack(blocks)  # (n_blocks, 128) token ids, -1 = pad
        block_expert = np.asarray(experts, dtype=np.int64)

    n_blocks = len(block_expert)
    n_blocks_core = n_blocks // NCORES
    per_core_experts = block_expert.reshape(NCORES, n_blocks_core)

    # Run-length encode each core's block->expert map; if all cores share the
    # same run-length pattern we can use compact per-run weight slots.
    def rle(v):
        runs = []
        for e in v:
            if runs and runs[-1][0] == e:
                runs[-1][1] += 1
            else:
                runs.append([int(e), 1])
        return runs

    core_runs = [rle(per_core_experts[c]) for c in range(NCORES)]
    lengths0 = [n for _, n in core_runs[0]]
    if all([n for _, n in core_runs[c]] == lengths0 for c in range(NCORES)):
        runs = [(s, n) for s, (_, n) in enumerate(core_runs[0])]
        slot_experts = [[e for e, _ in core_runs[c]] for c in range(NCORES)]
    else:
        runs = [(b, 1) for b in range(n_blocks_core)]
        slot_experts = [list(per_core_experts[c]) for c in range(NCORES)]
    n_slots = len(runs)

    # --- per-core inputs -------------------------------------------------------
    in_maps = []
    for c in range(NCORES):
        if block_tokens is None:
            rows = slice(c * n_blocks_core * PB, (c + 1) * n_blocks_core * PB)
            xc = x_bf[rows]
        else:
            tok = block_tokens[c * n_blocks_core : (c + 1) * n_blocks_core].ravel()
            xc = np.where(
                tok[:, None] >= 0, x_bf[np.maximum(tok, 0)], np.zeros((), BF16)
            ).astype(BF16)
        in_maps.append(
            {
                "xh": _host_layout_x(xc, runs),
                "w": _host_layout_w(w_bf[slot_experts[c]]),
            }
        )

    nc = _build_nc(n_blocks_core, runs, n_slots)
    res = run_bass_kernel_spmd(nc, in_maps, core_ids=list(range(NCORES)), trace=TRACE)
    LAST_RESULTS = res

    # --- unshard ---------------------------------------------------------------
    outs = [res.results[c]["out"] for c in range(NCORES)]
    if block_tokens is None:
        return np.concatenate(outs, axis=0)
    full = np.zeros((N_TOK, D_OUT), dtype=np.float32)
    flat_tok = block_tokens.ravel()
    flat_out = np.concatenate(outs, axis=0)
    valid = flat_tok >= 0
    full[flat_tok[valid]] = flat_out[valid]
    return full
